# revision 1
# baseline (speedup 1.0000x reference)
"""Trainium2 Bass kernel for nn_CustomLayer_22428319220577.

Math (reference):
    G    = Gmin + (W - Wmin) * a,  a = (Gmax-Gmin)/(Wmax-Wmin)
    G_q  = round((G-Gmin)/(Gmax-Gmin)*15) * (Gmax-Gmin)/15 + Gmin
    Geff = 1/(1/G_q + Rp*((M-i)+(j+1)))
    C    = x @ Geff ;  I = x @ G_q
    coeff= (rowrange I)/(rowrange C + EPS)
    C2   = (C - rowmean C)*coeff + rowmean I
    out  = (C2 - rowsum(x)*b)/a + bias,  b = Gmin - a*Wmin

Reformulated (removes the /a cancellation amplification):
    P  = G_q/a  - cP          (constant shift keeps PSUM magnitudes small;
    Q  = Geff/a - cQ           row ranges are shift-invariant)
    m  = rowmean_j(P) + cP - b/a   ( = (rowmean G_q - b)/a )
    mv = rowmean_j(Q)
    A = x@P ; B = x@Q ; [d|d2] = x@[m|mv]
    coeff = rowrange(A) / (rowrange(B) + EPS/a)
    out   = coeff*B + (d - coeff*d2) + bias

Sharding: data-parallel over batch. 8 cores, each takes 1024 rows of x,
replicates weight/bias (and the weight->Z precompute), no collectives.
"""
import os
import sys

sys.path.insert(0, "/opt/trn_rl_repo")

from contextlib import ExitStack

import numpy as np

import concourse.bass as bass
import concourse.tile as tile
from concourse import bacc, mybir
from concourse import bass_isa
from concourse.bass_utils import run_bass_kernel_spmd
from concourse.masks import make_identity

# problem constants (hardcoded per contract)
B_FULL, K, N = 8192, 1024, 1024
N_CORES = 8
B_SH = B_FULL // N_CORES          # 1024 rows per core
MT = B_SH // 128                  # 8 batch tiles per core
KB = K // 128                     # 8 k blocks

R_HRS, R_LRS, RP, BITS, EPS = 40000.0, 1000.0, 2.0, 4, 1e-8
GMIN, GMAX = 1.0 / R_HRS, 1.0 / R_LRS
LEVELS = float(2**BITS - 1)
GSPAN32 = np.float32(GMAX - GMIN)                   # fp32 of the python span
RSPANG = float(np.float32(1.0) / GSPAN32)           # 1/(Gmax-Gmin) in fp32
C2_IMM = float(np.float32(GSPAN32) / np.float32(LEVELS))
CP_SHIFT = 5.3                                      # ~mean of G_q/a
CQ_SHIFT = 2.2                                      # ~mean of Geff/a

FP32 = mybir.dt.float32
F32R = mybir.dt.float32r
I32 = mybir.dt.int32

# matmul operand dtype: FP32 = exact (4 cyc/row), F32R = ~11-bit mantissa (1 cyc/row)
MM_DT = FP32 if os.environ.get("KMM", "f32r") == "f32" else F32R


def _build():
    nc = bacc.Bacc("TRN2", target_bir_lowering=False, debug=False,
                   num_devices=N_CORES)

    xs = nc.dram_tensor("xs", [B_SH, K], FP32, kind="ExternalInput").ap()
    w = nc.dram_tensor("w", [K, N], FP32, kind="ExternalInput").ap()
    bias_d = nc.dram_tensor("bias", [N], FP32, kind="ExternalInput").ap()
    offs_d = nc.dram_tensor("offs", [128, KB], FP32, kind="ExternalInput").ap()
    out_d = nc.dram_tensor("out", [B_SH, N], FP32, kind="ExternalOutput").ap()

    AL = mybir.AluOpType

    def act_recip(dst, src):
        # raw ACT Reciprocal (~1.2e-5 maxrel on our ranges; bass's blanket
        # ban is for generic use). Frees the DVE of the reciprocal passes.
        eng = nc.scalar
        ins = [eng.lower_ap(src),
               mybir.ImmediateValue(dtype=mybir.dt.float32, value=0.0),
               mybir.ImmediateValue(dtype=mybir.dt.float32, value=1.0),
               mybir.ImmediateValue(dtype=mybir.dt.float32, value=0.0)]
        eng.add_instruction(mybir.InstActivation(
            name=nc.get_next_instruction_name(),
            func=mybir.ActivationFunctionType.Reciprocal,
            ins=ins, outs=[eng.lower_ap(dst)]))

    with tile.TileContext(nc) as tc, ExitStack() as ctx:
        consts = ctx.enter_context(tc.tile_pool(name="consts", bufs=1))
        wkeep = ctx.enter_context(tc.tile_pool(name="wkeep", bufs=1))
        wtiles = ctx.enter_context(tc.tile_pool(name="wtiles", bufs=2))
        stats = ctx.enter_context(tc.tile_pool(name="stats", bufs=1))
        xin = ctx.enter_context(tc.tile_pool(name="xin", bufs=3))
        xtsb = ctx.enter_context(tc.tile_pool(name="xtsb", bufs=2))
        bsb = ctx.enter_context(tc.tile_pool(name="bsb", bufs=2))
        outp = ctx.enter_context(tc.tile_pool(name="outp", bufs=2))
        mtst = ctx.enter_context(tc.tile_pool(name="mtst", bufs=8))
        ps_tr = ctx.enter_context(tc.tile_pool(name="ps_tr", bufs=2, space="PSUM"))
        ps_a = ctx.enter_context(tc.tile_pool(name="ps_a", bufs=1, space="PSUM"))
        ps_b = ctx.enter_context(tc.tile_pool(name="ps_b", bufs=1, space="PSUM"))
        ps_d = ctx.enter_context(tc.tile_pool(name="ps_d", bufs=2, space="PSUM"))

        # ---------- constants ----------
        ident = consts.tile([128, 128], FP32)
        make_identity(nc, ident[:])

        biasb = consts.tile([128, N], FP32)
        nc.sync.dma_start(
            out=biasb[:],
            in_=bass.AP(tensor=bias_d.tensor, offset=bias_d.offset,
                        ap=[[0, 128]] + bias_d.ap),
        )

        offs = consts.tile([128, KB], FP32)
        nc.sync.dma_start(out=offs[:], in_=offs_d)

        # Rpj[p, j] = RP*(j+1)  (same for all partitions)
        rpj_i = consts.tile([128, N], I32)
        nc.gpsimd.iota(rpj_i[:], pattern=[[1, N]], base=0, channel_multiplier=0)
        rpj = consts.tile([128, N], FP32)
        nc.vector.tensor_scalar(out=rpj[:], in0=rpj_i[:], scalar1=RP, scalar2=RP,
                                op0=AL.mult, op1=AL.add)

        # ---------- W load + global min/max ----------
        wkbs = []
        wmax8 = stats.tile([128, KB], FP32)
        wmin8 = stats.tile([128, KB], FP32)
        for kb in range(KB):
            wkb = wkeep.tile([128, N], FP32, tag=f"wkb{kb}")
            dma_eng = nc.sync if kb % 2 == 0 else nc.scalar
            dma_eng.dma_start(out=wkb[:], in_=w[kb * 128:(kb + 1) * 128, :])
            wkbs.append(wkb)
            nc.vector.tensor_reduce(out=wmax8[:, kb:kb + 1], in_=wkb[:],
                                    axis=mybir.AxisListType.X, op=AL.max)
            nc.vector.tensor_reduce(out=wmin8[:, kb:kb + 1], in_=wkb[:],
                                    axis=mybir.AxisListType.X, op=AL.min)

        wmaxp = stats.tile([128, 1], FP32)
        nc.vector.tensor_reduce(out=wmaxp[:], in_=wmax8[:],
                                axis=mybir.AxisListType.X, op=AL.max)
        wminp = stats.tile([128, 1], FP32)
        nc.vector.tensor_reduce(out=wminp[:], in_=wmin8[:],
                                axis=mybir.AxisListType.X, op=AL.min)
        wmax_t = stats.tile([128, 1], FP32)
        nc.gpsimd.partition_all_reduce(wmax_t[:], wmaxp[:], channels=128,
                                       reduce_op=bass_isa.ReduceOp.max)
        wminn = stats.tile([128, 1], FP32)
        nc.vector.tensor_scalar_mul(wminn[:], wminp[:], -1.0)
        wminn_t = stats.tile([128, 1], FP32)
        nc.gpsimd.partition_all_reduce(wminn_t[:], wminn[:], channels=128,
                                       reduce_op=bass_isa.ReduceOp.max)
        wmin_t = stats.tile([128, 1], FP32)
        nc.vector.tensor_scalar_mul(wmin_t[:], wminn_t[:], -1.0)

        # scalar tiles ([128,1] broadcast)
        span = stats.tile([128, 1], FP32)
        nc.vector.tensor_tensor(out=span[:], in0=wmax_t[:], in1=wmin_t[:],
                                op=AL.subtract)
        rspan_t = stats.tile([128, 1], FP32)
        nc.vector.reciprocal(rspan_t[:], span[:])
        aG_t = stats.tile([128, 1], FP32)   # a = (Gmax-Gmin) * (1/span)
        nc.vector.tensor_scalar_mul(aG_t[:], rspan_t[:], float(GSPAN32))
        inva_t = stats.tile([128, 1], FP32)  # 1/a = span * (1/(Gmax-Gmin))
        nc.vector.tensor_scalar_mul(inva_t[:], span[:], RSPANG)
        b_t = stats.tile([128, 1], FP32)     # b = Gmin - a*Wmin
        nc.vector.tensor_tensor(out=b_t[:], in0=aG_t[:], in1=wmin_t[:],
                                op=AL.mult)
        nc.vector.tensor_scalar(out=b_t[:], in0=b_t[:], scalar1=-1.0, scalar2=GMIN,
                                op0=AL.mult, op1=AL.add)
        eps_t = stats.tile([128, 1], FP32)   # EPS/a
        nc.vector.tensor_scalar_mul(eps_t[:], inva_t[:], EPS)
        binva_t = stats.tile([128, 1], FP32)  # b/a
        nc.vector.tensor_tensor(out=binva_t[:], in0=b_t[:], in1=inva_t[:],
                                op=AL.mult)
        negwmin = stats.tile([128, 1], FP32)
        nc.vector.tensor_scalar_mul(negwmin[:], wmin_t[:], -1.0)
        gmin_c = stats.tile([128, 1], FP32)
        nc.vector.memset(gmin_c[:], GMIN)

        # ---------- per-k-block precompute: Z = [P | Q], m ----------
        zsb = consts.tile([128, KB, 2 * N], MM_DT)
        m8 = consts.tile([128, KB, 2], MM_DT)
        for kb in range(KB):
            wkb = wkbs[kb]
            # quantization chain, bit-mirroring the reference fp32 op order:
            # y = (W - Wmin)*a ; G = Gmin + y ; gs = G - Gmin ;
            # t15 = (gs*(1/(Gmax-Gmin)))*15 ; r = rne(t15) ; gq = r*C2 + Gmin
            y = wtiles.tile([128, N], FP32, tag="y")
            nc.vector.tensor_scalar(out=y[:], in0=wkb[:], scalar1=wmin_t[:],
                                    scalar2=aG_t[:], op0=AL.subtract, op1=AL.mult)
            gsub = wtiles.tile([128, N], FP32, tag="gsub")
            nc.vector.tensor_scalar(out=gsub[:], in0=y[:], scalar1=GMIN,
                                    scalar2=GMIN, op0=AL.add, op1=AL.subtract)
            t15 = wtiles.tile([128, N], I32, tag="t15")
            nc.vector.tensor_scalar(out=t15[:], in0=gsub[:], scalar1=RSPANG,
                                    scalar2=LEVELS, op0=AL.mult, op1=AL.mult)
            gq = wtiles.tile([128, N], FP32, tag="gq")
            nc.scalar.activation(out=gq[:], in_=t15[:],
                                 func=mybir.ActivationFunctionType.Identity,
                                 bias=gmin_c[:], scale=C2_IMM)
            # P = gq*inva - cP  (ACT pass; accum gives rowsum for m)
            accP = mtst.tile([128, 1], FP32, tag="accP")
            nc.scalar.activation(out=zsb[:, kb, 0:N], in_=gq[:],
                                 func=mybir.ActivationFunctionType.Copy,
                                 bias=-CP_SHIFT, scale=inva_t[:],
                                 accum_out=accP[:])
            # m[kb] = accP/N + cP - b/a
            mtmp = mtst.tile([128, 1], FP32, tag="mtmp")
            nc.vector.tensor_scalar(out=mtmp[:], in0=accP[:], scalar1=1.0 / N,
                                    scalar2=CP_SHIFT, op0=AL.mult, op1=AL.add)
            nc.vector.tensor_tensor(out=m8[:, kb, 0:1], in0=mtmp[:],
                                    in1=binva_t[:], op=AL.subtract)
            # Geff = 1/(1/gq + Rp*((M-i)+(j+1)))
            inv = wtiles.tile([128, N], FP32, tag="inv")
            nc.vector.reciprocal_approx_fast(inv[:], gq[:])
            den = wtiles.tile([128, N], FP32, tag="den")
            nc.vector.affine_then_add(den[:], inv[:], rpj[:], 1.0,
                                      offs[:, kb:kb + 1])
            geff = wtiles.tile([128, N], FP32, tag="geff")
            act_recip(geff[:], den[:])
            # Q = geff*inva - cQ ; mv[kb] = rowmean(Q)
            accQ = mtst.tile([128, 1], FP32, tag="accQ")
            nc.scalar.activation(out=zsb[:, kb, N:2 * N], in_=geff[:],
                                 func=mybir.ActivationFunctionType.Copy,
                                 bias=-CQ_SHIFT, scale=inva_t[:],
                                 accum_out=accQ[:])
            nc.vector.tensor_scalar(out=m8[:, kb, 1:2], in0=accQ[:],
                                    scalar1=1.0 / N, scalar2=None, op0=AL.mult)

        # ---------- main loop over batch tiles ----------
        for mt in range(MT):
            xnat = xin.tile([128, K], FP32)
            xq = nc.scalar if mt % 2 == 0 else nc.sync
            xq.dma_start(out=xnat[:], in_=xs[mt * 128:(mt + 1) * 128, :])

            xt = xtsb.tile([128, K], MM_DT)
            for half in range(2):
                ptr = ps_tr.tile([128, 512], FP32)
                for q in range(4):
                    c = half * 4 + q
                    nc.tensor.transpose(ptr[:, q * 128:(q + 1) * 128],
                                        xnat[:, c * 128:(c + 1) * 128], ident[:])
                nc.scalar.copy(xt[:, half * 512:(half + 1) * 512], ptr[:])

            pa = ps_a.tile([128, 2, 512], FP32)
            pb = ps_b.tile([128, 2, 512], FP32)
            pd = ps_d.tile([128, 2], FP32)
            for kb in range(KB):
                lhsT = xt[:, kb * 128:(kb + 1) * 128]
                st, sp = kb == 0, kb == KB - 1
                nc.tensor.matmul(pa[:, 0, :], lhsT, zsb[:, kb, 0:512],
                                 start=st, stop=sp)
                nc.tensor.matmul(pa[:, 1, :], lhsT, zsb[:, kb, 512:1024],
                                 start=st, stop=sp)
                nc.tensor.matmul(pb[:, 0, :], lhsT, zsb[:, kb, 1024:1536],
                                 start=st, stop=sp)
                nc.tensor.matmul(pb[:, 1, :], lhsT, zsb[:, kb, 1536:2048],
                                 start=st, stop=sp)
                nc.tensor.matmul(pd[:], lhsT, m8[:, kb, :],
                                 start=st, stop=sp)

            # ranges of A directly from PSUM
            amax = mtst.tile([128, 1], FP32, tag="amax")
            nc.vector.tensor_reduce(out=amax[:], in_=pa[:], axis=mybir.AxisListType.XY,
                                    op=AL.max)
            amin = mtst.tile([128, 1], FP32, tag="amin")
            nc.vector.tensor_reduce(out=amin[:], in_=pa[:], axis=mybir.AxisListType.XY,
                                    op=AL.min)
            # copy B and [d|d2] to SBUF (frees PSUM)
            bs = bsb.tile([128, N], FP32)
            nc.scalar.copy(bs[:, 0:512], pb[:, 0, :])
            nc.scalar.copy(bs[:, 512:1024], pb[:, 1, :])
            dsb = mtst.tile([128, 2], FP32, tag="dsb")
            nc.scalar.copy(dsb[:], pd[:])

            bmax = mtst.tile([128, 1], FP32, tag="bmax")
            nc.vector.tensor_reduce(out=bmax[:], in_=bs[:], axis=mybir.AxisListType.X,
                                    op=AL.max)
            bmin = mtst.tile([128, 1], FP32, tag="bmin")
            nc.vector.tensor_reduce(out=bmin[:], in_=bs[:], axis=mybir.AxisListType.X,
                                    op=AL.min)

            ra = mtst.tile([128, 1], FP32, tag="ra")
            nc.vector.tensor_tensor(out=ra[:], in0=amax[:], in1=amin[:],
                                    op=AL.subtract)
            rbe = mtst.tile([128, 1], FP32, tag="rbe")
            nc.vector.tensor_scalar(out=rbe[:], in0=bmax[:], scalar1=bmin[:],
                                    scalar2=eps_t[:], op0=AL.subtract, op1=AL.add)
            rc = mtst.tile([128, 1], FP32, tag="rc")
            nc.vector.reciprocal(rc[:], rbe[:])
            coeff = mtst.tile([128, 1], FP32, tag="coeff")
            nc.vector.tensor_tensor(out=coeff[:], in0=ra[:], in1=rc[:],
                                    op=AL.mult)
            # dcomb = d - coeff*d2
            cd2 = mtst.tile([128, 1], FP32, tag="cd2")
            nc.vector.tensor_tensor(out=cd2[:], in0=coeff[:], in1=dsb[:, 1:2],
                                    op=AL.mult)
            dcomb = mtst.tile([128, 1], FP32, tag="dcomb")
            nc.vector.tensor_tensor(out=dcomb[:], in0=dsb[:, 0:1], in1=cd2[:],
                                    op=AL.subtract)

            # out = (B*coeff + dcomb) + bias
            osb = outp.tile([128, N], FP32)
            nc.vector.affine_then_add(osb[:], bs[:], biasb[:], coeff[:], dcomb[:])
            oq = nc.sync if mt % 2 == 0 else nc.scalar
            oq.dma_start(out=out_d[mt * 128:(mt + 1) * 128, :], in_=osb[:])

    nc.compile()
    return nc


_NC_CACHE = None


def _get_nc():
    global _NC_CACHE
    if _NC_CACHE is None:
        _NC_CACHE = _build()
    return _NC_CACHE


def _offs_np():
    p = np.arange(128, dtype=np.float64)[:, None]
    kb = np.arange(KB, dtype=np.float64)[None, :]
    return (RP * (K - (kb * 128 + p))).astype(np.float32)


def kernel(x, weight, bias):
    x = np.ascontiguousarray(x, np.float32)
    weight = np.ascontiguousarray(weight, np.float32)
    bias = np.ascontiguousarray(bias, np.float32)
    nc = _get_nc()
    offs = _offs_np()
    in_maps = [
        {"xs": x[c * B_SH:(c + 1) * B_SH], "w": weight, "bias": bias, "offs": offs}
        for c in range(N_CORES)
    ]
    res = run_bass_kernel_spmd(nc, in_maps, core_ids=list(range(N_CORES)))
    return np.concatenate([res.results[c]["out"] for c in range(N_CORES)], axis=0)



# revision 4
# speedup vs baseline: 1.1053x; 1.1053x over previous
"""Trainium2 Bass kernel for nn_CustomLayer_22428319220577 (v2).

Math (reference):
    G    = Gmin + (W - Wmin) * a,  a = (Gmax-Gmin)/(Wmax-Wmin)
    G_q  = round((G-Gmin)/(Gmax-Gmin)*15) * (Gmax-Gmin)/15 + Gmin
    Geff = 1/(1/G_q + Rp*((M-i)+(j+1)))
    C    = x @ Geff ;  I = x @ G_q
    coeff= (rowrange I)/(rowrange C + EPS)
    C2   = (C - rowmean C)*coeff + rowmean I
    out  = (C2 - rowsum(x)*b)/a + bias,  b = Gmin - a*Wmin

v2 formulation (current-space matmuls, no P/Q scale passes):
    Gc = G_q - cG            (cG = mid of conductance range; ranges are
    H  = Geff                 shift-invariant so coeff is unchanged)
    u  = (rowmean_j G_q - b)/a ;  mv = rowmean_j Geff
    A' = x@Gc (+ d-cols u,mv) ;  B' = x@H
    coeff = rowrange(A') / (rowrange(B') + EPS)
    out   = coeff/a * (B' - x@mv) + x@u + bias

Layout per k-block kb of zsb [128, KB, 2048] (f32r):
    cols    0:1022  Gc (columns 1022,1023 of G_q are dropped from the
                    A-range — u/mv overwrite them so the d-terms ride in
                    the A accumulation group's last PSUM bank)
    col  1022       u ;  col 1023  mv
    cols 1024:2048  H

Sharding: data-parallel over batch; 8 cores x 1024 rows, weight precompute
replicated, no collectives.
"""
import os
import sys

sys.path.insert(0, "/opt/trn_rl_repo")

DEBUG_DUMPS = os.environ.get("KDBG", "") == "1"

from contextlib import ExitStack

import numpy as np

import concourse.bass as bass
import concourse.tile as tile
from concourse import bacc, mybir
from concourse import bass_isa
from concourse.bass_utils import run_bass_kernel_spmd
from concourse.masks import make_identity

# problem constants (hardcoded per contract)
B_FULL, K, N = 8192, 1024, 1024
N_CORES = 8
B_SH = B_FULL // N_CORES          # 1024 rows per core
MT = B_SH // 128                  # 8 batch tiles per core
KB = K // 128                     # 8 k blocks

R_HRS, R_LRS, RP, BITS, EPS = 40000.0, 1000.0, 2.0, 4, 1e-8
GMIN, GMAX = 1.0 / R_HRS, 1.0 / R_LRS
GSPAN32 = np.float32(GMAX) - np.float32(GMIN)
RSPANG = float(np.float32(1.0) / GSPAN32)           # 1/(Gmax-Gmin) in fp32
LEVELS = float(2**BITS - 1)
C2_IMM = float(np.float32(GSPAN32) / np.float32(LEVELS))
CG = float(np.float32(0.5) * (np.float32(GMIN) + np.float32(GMAX)))
GMC = float(np.float32(GMIN) - np.float32(CG))      # Gmin - cG
FLT_MAX = 3.4028234663852886e38

FP32 = mybir.dt.float32
F32R = mybir.dt.float32r
I32 = mybir.dt.int32
MM_DT = F32R


def _build():
    nc = bacc.Bacc("TRN2", target_bir_lowering=False, debug=False,
                   num_devices=N_CORES)

    xs = nc.dram_tensor("xs", [B_SH, K], FP32, kind="ExternalInput").ap()
    w = nc.dram_tensor("w", [K, N], FP32, kind="ExternalInput").ap()
    bias_d = nc.dram_tensor("bias", [N], FP32, kind="ExternalInput").ap()
    offs_d = nc.dram_tensor("offs", [128, KB], FP32, kind="ExternalInput").ap()
    out_d = nc.dram_tensor("out", [B_SH, N], FP32, kind="ExternalOutput").ap()
    if DEBUG_DUMPS:
        dbg_s = nc.dram_tensor("dbg_s", [128, 8], FP32, kind="ExternalOutput").ap()
        dbg_j = nc.dram_tensor("dbg_j", [128, N], FP32, kind="ExternalOutput").ap()
        dbg_t = nc.dram_tensor("dbg_t", [128, N], I32, kind="ExternalOutput").ap()
        dbg_i = nc.dram_tensor("dbg_i", [128, N], FP32, kind="ExternalOutput").ap()
        dbg_d = nc.dram_tensor("dbg_d", [128, N], FP32, kind="ExternalOutput").ap()
        dbg_z = nc.dram_tensor("dbg_z", [128, 2 * N], FP32, kind="ExternalOutput").ap()

    AL = mybir.AluOpType
    AX = mybir.AxisListType

    def act_raw(dst, src, bias=0.0, scale=1.0, accum=None):
        # raw ACT Reciprocal (~1.2e-5 maxrel on our ranges; bass's blanket
        # ban is for generic use): dst = 1/(src*scale + bias)
        eng = nc.scalar
        ins = [eng.lower_ap(src),
               mybir.ImmediateValue(dtype=mybir.dt.float32, value=float(bias)),
               mybir.ImmediateValue(dtype=mybir.dt.float32, value=float(scale)),
               mybir.ImmediateValue(dtype=mybir.dt.float32, value=0.0)]
        outs = [eng.lower_ap(dst)]
        if accum is not None:
            outs.append(eng.lower_ap(accum))
        eng.add_instruction(mybir.InstActivation(
            name=nc.get_next_instruction_name(),
            func=mybir.ActivationFunctionType.Reciprocal,
            ins=ins, outs=outs))

    with tile.TileContext(nc) as tc, ExitStack() as ctx:
        consts = ctx.enter_context(tc.tile_pool(name="consts", bufs=1))
        wk = ctx.enter_context(tc.tile_pool(name="wk", bufs=1))
        zp = ctx.enter_context(tc.tile_pool(name="zp", bufs=1))
        stats = ctx.enter_context(tc.tile_pool(name="stats", bufs=1))
        scrp = ctx.enter_context(tc.tile_pool(name="scrp", bufs=2))
        wtp = ctx.enter_context(tc.tile_pool(name="wtp", bufs=2))
        xin = ctx.enter_context(tc.tile_pool(name="xin", bufs=2))
        xtk = ctx.enter_context(tc.tile_pool(name="xtk", bufs=1))
        ob1 = ctx.enter_context(tc.tile_pool(name="ob1", bufs=1))
        bsp = ctx.enter_context(tc.tile_pool(name="bsp", bufs=2))
        obp = ctx.enter_context(tc.tile_pool(name="obp", bufs=2))
        smp = ctx.enter_context(tc.tile_pool(name="smp", bufs=2))
        ps_tr = ctx.enter_context(tc.tile_pool(name="ps_tr", bufs=2, space="PSUM"))
        pap = ctx.enter_context(tc.tile_pool(name="pap", bufs=2, space="PSUM"))
        pbp = ctx.enter_context(tc.tile_pool(name="pbp", bufs=1, space="PSUM"))

        # ---------- phase 0: constants, W load + global min/max ----------
        ident = consts.tile([128, 128], FP32)
        make_identity(nc, ident[:])

        biasb = consts.tile([128, N], FP32)
        nc.sync.dma_start(
            out=biasb[:],
            in_=bass.AP(tensor=bias_d.tensor, offset=bias_d.offset,
                        ap=[[0, 128]] + bias_d.ap),
        )
        offs = consts.tile([128, KB], FP32)
        nc.sync.dma_start(out=offs[:], in_=offs_d)

        # jif[p, j] = RP*(j - p)  (exact in fp32: RP=2, |j-p|<2048)
        jii = consts.tile([128, N], I32)
        nc.gpsimd.iota(jii[:], pattern=[[1, N]], base=0, channel_multiplier=-1)
        jif = consts.tile([128, N], FP32)
        nc.gpsimd.tensor_scalar(out=jif[:], in0=jii[:], scalar1=RP, scalar2=None,
                                op0=AL.mult)

        w8 = wk.tile([128, KB, N], FP32)
        wmax8 = stats.tile([128, KB], FP32)
        wmin8 = stats.tile([128, KB], FP32)
        for kb in range(KB):
            dq = nc.sync if kb % 2 == 0 else nc.scalar
            dq.dma_start(out=w8[:, kb, :], in_=w[kb * 128:(kb + 1) * 128, :])
            nc.vector.tensor_reduce(out=wmax8[:, kb:kb + 1], in_=w8[:, kb, :],
                                    axis=AX.X, op=AL.max)
            nc.vector.tensor_reduce(out=wmin8[:, kb:kb + 1], in_=w8[:, kb, :],
                                    axis=AX.X, op=AL.min)

        # pack [wmax | -wmin] and one partition all-reduce(max)
        pk = stats.tile([128, 2], FP32)
        nc.vector.tensor_reduce(out=pk[:, 0:1], in_=wmax8[:], axis=AX.X, op=AL.max)
        wminp = stats.tile([128, 1], FP32)
        nc.vector.tensor_reduce(out=wminp[:], in_=wmin8[:], axis=AX.X, op=AL.min)
        nc.vector.tensor_scalar(out=pk[:, 1:2], in0=wminp[:], scalar1=-1.0,
                                scalar2=None, op0=AL.mult)
        pkt = stats.tile([128, 2], FP32)
        nc.gpsimd.partition_all_reduce(pkt[:], pk[:], channels=128,
                                       reduce_op=bass_isa.ReduceOp.max)
        wmax_t = pkt[:, 0:1]
        wmin_t = stats.tile([128, 1], FP32)
        nc.vector.tensor_scalar(out=wmin_t[:], in0=pkt[:, 1:2], scalar1=-1.0,
                                scalar2=None, op0=AL.mult)

        span = stats.tile([128, 1], FP32)
        nc.vector.tensor_scalar(out=span[:], in0=wmax_t, scalar1=wmin_t[:],
                                scalar2=None, op0=AL.subtract)
        rspan = stats.tile([128, 1], FP32)
        nc.vector.reciprocal(rspan[:], span[:])
        s15_t = stats.tile([128, 1], FP32)      # 15/span
        nc.vector.tensor_scalar(out=s15_t[:], in0=rspan[:], scalar1=LEVELS,
                                scalar2=None, op0=AL.mult)
        aG_t = stats.tile([128, 1], FP32)       # a
        nc.vector.tensor_scalar(out=aG_t[:], in0=rspan[:], scalar1=float(GSPAN32),
                                scalar2=None, op0=AL.mult)
        inva_t = stats.tile([128, 1], FP32)     # 1/a
        nc.vector.tensor_scalar(out=inva_t[:], in0=span[:], scalar1=RSPANG,
                                scalar2=None, op0=AL.mult)
        b_t = stats.tile([128, 1], FP32)        # b = Gmin - a*Wmin
        nc.vector.tensor_tensor(out=b_t[:], in0=aG_t[:], in1=wmin_t[:],
                                op=AL.mult)
        nc.vector.tensor_scalar(out=b_t[:], in0=b_t[:], scalar1=-1.0,
                                scalar2=GMIN, op0=AL.mult, op1=AL.add)
        gmc_c = stats.tile([128, 1], FP32)      # Gmin - cG bias tile
        nc.vector.memset(gmc_c[:], GMC)

        if DEBUG_DUMPS:
            dbgs = stats.tile([128, 8], FP32)
            for di, src in enumerate([wmax_t, wmin_t[:], span[:], rspan[:],
                                      s15_t[:], aG_t[:], inva_t[:], b_t[:]]):
                nc.scalar.copy(dbgs[:, di:di + 1], src)
            nc.sync.dma_start(out=dbg_s, in_=dbgs[:])
            nc.sync.dma_start(out=dbg_j, in_=jif[:])

        # ---------- phase 1: per-k-block zsb = [Gc|u|mv|H] ----------
        zsb = zp.tile([128, KB, 2 * N], MM_DT)
        acc1 = stats.tile([128, KB], FP32)
        accQ = stats.tile([128, KB], FP32)
        for kb in range(KB):
            t15 = wtp.tile([128, N], I32, tag="t15")
            nc.vector.tensor_scalar(out=t15[:], in0=w8[:, kb, :],
                                    scalar1=wmin_t[:], scalar2=s15_t[:],
                                    op0=AL.subtract, op1=AL.mult)
            # Gc = rne(t15)*C2 + (Gmin - cG); accum -> rowsum for u
            nc.scalar.activation(out=zsb[:, kb, 0:N], in_=t15[:],
                                 func=mybir.ActivationFunctionType.Identity,
                                 bias=gmc_c[:], scale=C2_IMM,
                                 accum_out=acc1[:, kb:kb + 1])
            # inv = 1/G_q  (from the exact int levels)
            inv = wtp.tile([128, N], FP32, tag="inv")
            act_raw(inv[:], t15[:], bias=GMIN, scale=C2_IMM)
            # den = 1/G_q + Rp*((M-i)+(j+1)) = (jif + offs_kb) + inv, exact
            den = wtp.tile([128, N], FP32, tag="den")
            nc.vector.affine_then_add(den[:], jif[:], inv[:], 1.0,
                                      offs[:, kb:kb + 1])
            # H = Geff; accum -> rowsum for mv
            act_raw(zsb[:, kb, N:2 * N], den[:], bias=0.0, scale=1.0,
                    accum=accQ[:, kb:kb + 1])
            # u = ((acc1/N + cG) - b)/a  -> col 1022 ; mv = accQ/N -> col 1023
            macc = smp.tile([128, 1], FP32, tag="macc")
            nc.vector.tensor_scalar(out=macc[:], in0=acc1[:, kb:kb + 1],
                                    scalar1=1.0 / N, scalar2=CG,
                                    op0=AL.mult, op1=AL.add)
            nc.vector.tensor_scalar(out=zsb[:, kb, N - 2:N - 1], in0=macc[:],
                                    scalar1=b_t[:], scalar2=inva_t[:],
                                    op0=AL.subtract, op1=AL.mult)
            nc.vector.tensor_scalar(out=zsb[:, kb, N - 1:N], in0=accQ[:, kb:kb + 1],
                                    scalar1=1.0 / N, scalar2=None, op0=AL.mult)
            if DEBUG_DUMPS and kb == 0:
                nc.sync.dma_start(out=dbg_t, in_=t15[:])
                nc.sync.dma_start(out=dbg_i, in_=inv[:])
                nc.sync.dma_start(out=dbg_d, in_=den[:])
        if DEBUG_DUMPS:
            nc.sync.dma_start(out=dbg_z, in_=zsb[:, 0, :].bitcast(FP32))

        # ---------- x pipeline: dma, transpose (f32r), copy to xt ----------
        xt = xtk.tile([128, MT, K], MM_DT)
        for mt in range(MT):
            xnat = xin.tile([128, K], FP32)
            nc.sync.dma_start(out=xnat[:], in_=xs[mt * 128:(mt + 1) * 128, :])
            for half in range(2):
                ptr = ps_tr.tile([128, 512], FP32)
                for q in range(4):
                    c = half * 4 + q
                    nc.tensor.transpose(ptr[:, q * 128:(q + 1) * 128],
                                        xnat[:, c * 128:(c + 1) * 128],
                                        ident[:])
                if mt % 2 == 0:
                    nc.vector.tensor_copy(xt[:, mt, half * 512:(half + 1) * 512],
                                          ptr[:])
                else:
                    nc.scalar.copy(xt[:, mt, half * 512:(half + 1) * 512],
                                   ptr[:])

        # ---------- matmuls + per-mt drains ----------
        pas, pbs = {}, {}

        def emit_A(mt):
            pa = pap.tile([128, N], FP32, tag="pa")
            pas[mt] = pa
            for kb in range(KB):
                lhsT = xt[:, mt, kb * 128:(kb + 1) * 128]
                st, sp = kb == 0, kb == KB - 1
                nc.tensor.matmul(pa[:, 0:512], lhsT, zsb[:, kb, 0:512],
                                 start=st, stop=sp)
                nc.tensor.matmul(pa[:, 512:1024], lhsT, zsb[:, kb, 512:1024],
                                 start=st, stop=sp)

        def emit_B(mt):
            pb = pbp.tile([128, N], FP32, tag="pb")
            pbs[mt] = pb
            for kb in range(KB):
                lhsT = xt[:, mt, kb * 128:(kb + 1) * 128]
                st, sp = kb == 0, kb == KB - 1
                nc.tensor.matmul(pb[:, 0:512], lhsT, zsb[:, kb, N:N + 512],
                                 start=st, stop=sp)
                nc.tensor.matmul(pb[:, 512:1024], lhsT, zsb[:, kb, N + 512:2 * N],
                                 start=st, stop=sp)

        def emit_drain(mt):
            pa, pb = pas[mt], pbs[mt]
            amax = smp.tile([128, 1], FP32, tag="amax")
            amin = smp.tile([128, 1], FP32, tag="amin")
            bmax = smp.tile([128, 1], FP32, tag="bmax")
            bmin = smp.tile([128, 1], FP32, tag="bmin")
            # A stats straight off PSUM (single-PSUM-input rule)
            nc.vector.tensor_reduce(out=amax[:], in_=pa[:, 0:1022], axis=AX.X,
                                    op=AL.max)
            nc.vector.tensor_reduce(out=amin[:], in_=pa[:, 0:1022], axis=AX.X,
                                    op=AL.min)
            # B: copy to SBUF (frees pb bank), then stats off SBUF
            bs = bsp.tile([128, N], FP32, tag="bs")
            nc.scalar.copy(bs[:], pb[:])
            nc.vector.tensor_reduce(out=bmax[:], in_=bs[:], axis=AX.X, op=AL.max)
            nc.vector.tensor_reduce(out=bmin[:], in_=bs[:], axis=AX.X, op=AL.min)
            ra = smp.tile([128, 1], FP32, tag="ra")
            nc.vector.tensor_tensor(out=ra[:], in0=amax[:], in1=amin[:],
                                    op=AL.subtract)
            rbe = smp.tile([128, 1], FP32, tag="rbe")
            nc.vector.tensor_scalar(out=rbe[:], in0=bmax[:], scalar1=bmin[:],
                                    scalar2=EPS, op0=AL.subtract, op1=AL.add)
            rc = smp.tile([128, 1], FP32, tag="rc")
            nc.vector.reciprocal(rc[:], rbe[:])
            coeff = smp.tile([128, 1], FP32, tag="coeff")
            nc.vector.tensor_tensor(out=coeff[:], in0=ra[:], in1=rc[:],
                                    op=AL.mult)
            ci = smp.tile([128, 1], FP32, tag="ci")
            nc.vector.tensor_scalar(out=ci[:], in0=coeff[:], scalar1=inva_t[:],
                                    scalar2=None, op0=AL.mult)
            cd2 = smp.tile([128, 1], FP32, tag="cd2")
            nc.vector.tensor_tensor(out=cd2[:], in0=ci[:], in1=pa[:, 1023:1024],
                                    op=AL.mult)
            dcomb = smp.tile([128, 1], FP32, tag="dcomb")
            nc.vector.tensor_tensor(out=dcomb[:], in0=pa[:, 1022:1023],
                                    in1=cd2[:], op=AL.subtract)
            osb = obp.tile([128, N], FP32, tag="osb")
            if mt < MT - 2:
                # out = (bs*ci + dcomb) on ACT, + bias on gpsimd
                osb1 = ob1.tile([128, N], FP32, tag="osb1")
                nc.scalar.activation(out=osb1[:], in_=bs[:],
                                     func=mybir.ActivationFunctionType.Identity,
                                     bias=dcomb[:], scale=ci[:])
                nc.gpsimd.tensor_tensor(out=osb[:], in0=osb1[:], in1=biasb[:],
                                        op=AL.add)
            else:
                # tail mts: single DVE pass for lower latency
                nc.vector.affine_then_add(osb[:], bs[:], biasb[:], ci[:],
                                          dcomb[:])
            nc.scalar.dma_start(out=out_d[mt * 128:(mt + 1) * 128, :], in_=osb[:])

        emit_A(0)
        emit_B(0)
        emit_A(1)
        for mt in range(MT):
            emit_drain(mt)
            if mt + 1 < MT:
                emit_B(mt + 1)
            if mt + 2 < MT:
                emit_A(mt + 2)

    nc.compile()
    return nc


_NC_CACHE = None


def _get_nc():
    global _NC_CACHE
    if _NC_CACHE is None:
        _NC_CACHE = _build()
    return _NC_CACHE


def _offs_np():
    # offs[p, kb] = Rp*(K + 1 - kb*128), constant over p (the i-dependence
    # rides in jif = Rp*(j - p) on device; sum reproduces Rp*((M-i)+(j+1)))
    kb = np.arange(KB, dtype=np.float64)[None, :]
    p = np.zeros((128, 1), dtype=np.float64)
    return (RP * (K + 1.0 - kb * 128.0) + p).astype(np.float32)


def kernel(x, weight, bias):
    x = np.ascontiguousarray(x, np.float32)
    weight = np.ascontiguousarray(weight, np.float32)
    bias = np.ascontiguousarray(bias, np.float32)
    nc = _get_nc()
    offs = _offs_np()
    in_maps = [
        {"xs": x[c * B_SH:(c + 1) * B_SH], "w": weight, "bias": bias, "offs": offs}
        for c in range(N_CORES)
    ]
    res = run_bass_kernel_spmd(nc, in_maps, core_ids=list(range(N_CORES)))
    return np.concatenate([res.results[c]["out"] for c in range(N_CORES)], axis=0)


# revision 5
# speedup vs baseline: 1.1903x; 1.0769x over previous
"""Trainium2 Bass kernel for nn_CustomLayer_22428319220577 (v3).

Math (reference):
    G    = Gmin + (W - Wmin) * a,  a = (Gmax-Gmin)/(Wmax-Wmin)
    G_q  = round((G-Gmin)/(Gmax-Gmin)*15) * (Gmax-Gmin)/15 + Gmin
    Geff = 1/(1/G_q + Rp*((M-i)+(j+1)))
    C    = x @ Geff ;  I = x @ G_q
    coeff= (rowrange I)/(rowrange C + EPS)
    C2   = (C - rowmean C)*coeff + rowmean I
    out  = (C2 - rowsum(x)*b)/a + bias,  b = Gmin - a*Wmin

v3 formulation (current-space matmuls):
    Gc = G_q - cG ;  H = Geff
    u  = (rowmean_j G_q - b)/a ;  mv = rowmean_j Geff
    A' = x@Gc (cols 1022/1023 replaced by the u/mv product columns)
    B' = x@H
    coeff = rowrange(A'[:, :1022]) / (rowrange(B') + EPS)
    out   = coeff/a * (B' - d2) + d1 + bias      (d1 = x@u, d2 = x@mv)

Schedule: W dma + split min/max (DVE reduces kb0-3, gpsimd pair-max trees
kb4-7) -> scalar chain -> staggered phase-1 pipeline (DVE t15/den, ACT
Gc/inv/H+accums) with A0/A1/B0 matmuls and x transposes riding along ->
drain pipeline with bf16 centered stats and ACT+gpsimd combine.
"""
import os
import sys

sys.path.insert(0, "/opt/trn_rl_repo")

from contextlib import ExitStack

import numpy as np

import concourse.bass as bass
import concourse.tile as tile
from concourse import bacc, mybir
from concourse import bass_isa
from concourse.bass_utils import run_bass_kernel_spmd
from concourse.masks import make_identity

B_FULL, K, N = 8192, 1024, 1024
N_CORES = 8
B_SH = B_FULL // N_CORES
MT = B_SH // 128
KB = K // 128

R_HRS, R_LRS, RP, BITS, EPS = 40000.0, 1000.0, 2.0, 4, 1e-8
GMIN, GMAX = 1.0 / R_HRS, 1.0 / R_LRS
GSPAN32 = np.float32(GMAX) - np.float32(GMIN)
RSPANG = float(np.float32(1.0) / GSPAN32)
LEVELS = float(2**BITS - 1)
C2_IMM = float(np.float32(GSPAN32) / np.float32(LEVELS))
CG = float(np.float32(0.5) * (np.float32(GMIN) + np.float32(GMAX)))
GMC = float(np.float32(GMIN) - np.float32(CG))

FP32 = mybir.dt.float32
F32R = mybir.dt.float32r
BF16 = mybir.dt.bfloat16
I32 = mybir.dt.int32
MM_DT = F32R


def _build():
    nc = bacc.Bacc("TRN2", target_bir_lowering=False, debug=False,
                   num_devices=N_CORES)

    xs = nc.dram_tensor("xs", [B_SH, K], FP32, kind="ExternalInput").ap()
    w = nc.dram_tensor("w", [K, N], FP32, kind="ExternalInput").ap()
    bias_d = nc.dram_tensor("bias", [N], FP32, kind="ExternalInput").ap()
    offs_d = nc.dram_tensor("offs", [128, KB], FP32, kind="ExternalInput").ap()
    out_d = nc.dram_tensor("out", [B_SH, N], FP32, kind="ExternalOutput").ap()

    AL = mybir.AluOpType
    AX = mybir.AxisListType

    def act_recip(dst, src, bias=0.0, scale=1.0, accum=None):
        # raw ACT Reciprocal: dst = 1/(src*scale + bias)
        eng = nc.scalar
        ins = [eng.lower_ap(src),
               mybir.ImmediateValue(dtype=mybir.dt.float32, value=float(bias)),
               mybir.ImmediateValue(dtype=mybir.dt.float32, value=float(scale)),
               mybir.ImmediateValue(dtype=mybir.dt.float32, value=0.0)]
        outs = [eng.lower_ap(dst)]
        if accum is not None:
            outs.append(eng.lower_ap(accum))
        eng.add_instruction(mybir.InstActivation(
            name=nc.get_next_instruction_name(),
            func=mybir.ActivationFunctionType.Reciprocal,
            ins=ins, outs=outs))

    with tile.TileContext(nc) as tc, ExitStack() as ctx:
        consts = ctx.enter_context(tc.tile_pool(name="consts", bufs=1))
        wk = ctx.enter_context(tc.tile_pool(name="wk", bufs=1))
        zp = ctx.enter_context(tc.tile_pool(name="zp", bufs=1))
        stats = ctx.enter_context(tc.tile_pool(name="stats", bufs=1))
        gtp = ctx.enter_context(tc.tile_pool(name="gtp", bufs=2))
        wtp = ctx.enter_context(tc.tile_pool(name="wtp", bufs=2))
        xin = ctx.enter_context(tc.tile_pool(name="xin", bufs=2))
        xtk = ctx.enter_context(tc.tile_pool(name="xtk", bufs=1))
        asp = ctx.enter_context(tc.tile_pool(name="asp", bufs=2))
        bsp = ctx.enter_context(tc.tile_pool(name="bsp", bufs=2))
        ob1 = ctx.enter_context(tc.tile_pool(name="ob1", bufs=1))
        obp = ctx.enter_context(tc.tile_pool(name="obp", bufs=2))
        smp = ctx.enter_context(tc.tile_pool(name="smp", bufs=2))
        ps_tr = ctx.enter_context(tc.tile_pool(name="ps_tr", bufs=2, space="PSUM"))
        pap = ctx.enter_context(tc.tile_pool(name="pap", bufs=2, space="PSUM"))
        pbp = ctx.enter_context(tc.tile_pool(name="pbp", bufs=1, space="PSUM"))

        # ---------- phase 0 ----------
        ident = consts.tile([128, 128], FP32)
        make_identity(nc, ident[:])
        # jif[p, j] = j - p (fp32, exact); RP scaling folded into the den op
        jif = consts.tile([128, N], FP32)
        nc.gpsimd.iota(jif[:], pattern=[[1, N]], base=0, channel_multiplier=-1,
                       allow_small_or_imprecise_dtypes=True)

        biasb = consts.tile([128, N], FP32)
        nc.sync.dma_start(
            out=biasb[:],
            in_=bass.AP(tensor=bias_d.tensor, offset=bias_d.offset,
                        ap=[[0, 128]] + bias_d.ap),
        )
        offs = consts.tile([128, KB], FP32)
        nc.sync.dma_start(out=offs[:], in_=offs_d)

        # W dma alternating queues; per-kb min/max reduces on DVE
        w8 = wk.tile([128, KB, N], FP32)
        wmax8 = stats.tile([128, KB], FP32)
        wmin8 = stats.tile([128, KB], FP32)
        for kb in range(KB):
            dq = nc.sync if kb % 2 == 0 else nc.scalar
            dq.dma_start(out=w8[:, kb, :], in_=w[kb * 128:(kb + 1) * 128, :])
            nc.vector.tensor_reduce(out=wmax8[:, kb:kb + 1], in_=w8[:, kb, :],
                                    axis=AX.X, op=AL.max)
            nc.vector.tensor_reduce(out=wmin8[:, kb:kb + 1], in_=w8[:, kb, :],
                                    axis=AX.X, op=AL.min)

        # pack [wmax | -wmin], one partition all-reduce(max)
        pk = stats.tile([128, 2], FP32)
        nc.vector.tensor_reduce(out=pk[:, 0:1], in_=wmax8[:], axis=AX.X, op=AL.max)
        wminp = stats.tile([128, 1], FP32)
        nc.vector.tensor_reduce(out=wminp[:], in_=wmin8[:], axis=AX.X, op=AL.min)
        nc.vector.tensor_scalar(out=pk[:, 1:2], in0=wminp[:], scalar1=-1.0,
                                scalar2=None, op0=AL.mult)
        pkt = stats.tile([128, 2], FP32)
        nc.gpsimd.partition_all_reduce(pkt[:], pk[:], channels=128,
                                       reduce_op=bass_isa.ReduceOp.max)

        # ---------- x dma + first two transposes ----------
        xt = xtk.tile([128, MT, K], MM_DT)
        xnats = {}
        for mt in range(MT):
            xnat = xin.tile([128, K], FP32, tag="xn")
            xnats[mt] = xnat
            nc.sync.dma_start(out=xnat[:], in_=xs[mt * 128:(mt + 1) * 128, :])

        def emit_tr(mt):
            xnat = xnats[mt]
            for half in range(2):
                ptr = ps_tr.tile([128, 512], FP32, tag="tr")
                for q in range(4):
                    c = half * 4 + q
                    nc.tensor.transpose(ptr[:, q * 128:(q + 1) * 128],
                                        xnat[:, c * 128:(c + 1) * 128], ident[:])
                yield ptr

        def emit_copy(mt, ptrs):
            for half, ptr in enumerate(ptrs):
                nc.vector.tensor_copy(xt[:, mt, half * 512:(half + 1) * 512],
                                      ptr[:])

        emit_copy(0, list(emit_tr(0)))
        emit_copy(1, list(emit_tr(1)))

        # ---------- scalar chain ----------
        wmax_t = pkt[:, 0:1]
        wmin_t = stats.tile([128, 1], FP32)
        nc.vector.tensor_scalar(out=wmin_t[:], in0=pkt[:, 1:2], scalar1=-1.0,
                                scalar2=None, op0=AL.mult)
        span = stats.tile([128, 1], FP32)
        nc.vector.tensor_scalar(out=span[:], in0=wmax_t, scalar1=wmin_t[:],
                                scalar2=None, op0=AL.subtract)
        rspan = stats.tile([128, 1], FP32)
        nc.vector.reciprocal(rspan[:], span[:])
        s15_t = stats.tile([128, 1], FP32)
        nc.vector.tensor_scalar(out=s15_t[:], in0=rspan[:], scalar1=LEVELS,
                                scalar2=None, op0=AL.mult)
        aG_t = stats.tile([128, 1], FP32)
        nc.vector.tensor_scalar(out=aG_t[:], in0=rspan[:], scalar1=float(GSPAN32),
                                scalar2=None, op0=AL.mult)
        inva_t = stats.tile([128, 1], FP32)
        nc.vector.tensor_scalar(out=inva_t[:], in0=span[:], scalar1=RSPANG,
                                scalar2=None, op0=AL.mult)
        b_t = stats.tile([128, 1], FP32)
        nc.vector.tensor_tensor(out=b_t[:], in0=aG_t[:], in1=wmin_t[:],
                                op=AL.mult)
        nc.vector.tensor_scalar(out=b_t[:], in0=b_t[:], scalar1=-1.0,
                                scalar2=GMIN, op0=AL.mult, op1=AL.add)
        gmc_c = stats.tile([128, 1], FP32)
        nc.vector.memset(gmc_c[:], GMC)

        # ---------- phase 1 (staggered) + A0/A1/B0 + transposes 2-7 ----------
        zsb = zp.tile([128, KB, 2 * N], MM_DT)
        acc1 = stats.tile([128, KB], FP32)
        accQ = stats.tile([128, KB], FP32)
        dens = {}

        pa0 = pap.tile([128, N], FP32, tag="pa")
        pa1 = pap.tile([128, N], FP32, tag="pa")
        pb0 = pbp.tile([128, N], FP32, tag="pb")
        pas = {0: pa0, 1: pa1}
        pbs = {0: pb0}

        def emit_quant(kb):
            t15 = wtp.tile([128, N], I32, tag="t15")
            nc.vector.tensor_scalar(out=t15[:], in0=w8[:, kb, :],
                                    scalar1=wmin_t[:], scalar2=s15_t[:],
                                    op0=AL.subtract, op1=AL.mult)
            nc.scalar.activation(out=zsb[:, kb, 0:N], in_=t15[:],
                                 func=mybir.ActivationFunctionType.Identity,
                                 bias=gmc_c[:], scale=C2_IMM,
                                 accum_out=acc1[:, kb:kb + 1])
            inv = wtp.tile([128, N], FP32, tag="inv")
            act_recip(inv[:], t15[:], bias=GMIN, scale=C2_IMM)
            den = wtp.tile([128, N], FP32, tag="den")
            # den = (jif*RP + offs_kb) + inv  (exact Rpar + 1/G_q)
            nc.vector.affine_then_add(den[:], jif[:], inv[:], RP,
                                      offs[:, kb:kb + 1])
            dens[kb] = den

        def emit_tail(kb):
            # H + accum, then u/mv into zsb cols 1022/1023
            act_recip(zsb[:, kb, N:2 * N], dens[kb][:], bias=0.0, scale=1.0,
                      accum=accQ[:, kb:kb + 1])
            macc = smp.tile([128, 1], FP32, tag="macc")
            nc.vector.tensor_scalar(out=macc[:], in0=acc1[:, kb:kb + 1],
                                    scalar1=1.0 / N, scalar2=CG,
                                    op0=AL.mult, op1=AL.add)
            nc.vector.tensor_scalar(out=zsb[:, kb, N - 2:N - 1], in0=macc[:],
                                    scalar1=b_t[:], scalar2=inva_t[:],
                                    op0=AL.subtract, op1=AL.mult)
            nc.vector.tensor_scalar(out=zsb[:, kb, N - 1:N],
                                    in0=accQ[:, kb:kb + 1],
                                    scalar1=1.0 / N, scalar2=None, op0=AL.mult)

        def emit_A_kb(mt, kb):
            pa = pas[mt]
            lhsT = xt[:, mt, kb * 128:(kb + 1) * 128]
            st, sp = kb == 0, kb == KB - 1
            nc.tensor.matmul(pa[:, 0:512], lhsT, zsb[:, kb, 0:512],
                             start=st, stop=sp)
            nc.tensor.matmul(pa[:, 512:1024], lhsT, zsb[:, kb, 512:1024],
                             start=st, stop=sp)

        def emit_B_kb(mt, kb):
            pb = pbs[mt]
            lhsT = xt[:, mt, kb * 128:(kb + 1) * 128]
            st, sp = kb == 0, kb == KB - 1
            nc.tensor.matmul(pb[:, 0:512], lhsT, zsb[:, kb, N:N + 512],
                             start=st, stop=sp)
            nc.tensor.matmul(pb[:, 512:1024], lhsT, zsb[:, kb, N + 512:2 * N],
                             start=st, stop=sp)

        for kb in range(KB):
            emit_quant(kb)
            if kb >= 1:
                emit_tail(kb - 1)
                emit_A_kb(0, kb - 1)
                emit_A_kb(1, kb - 1)
                emit_B_kb(0, kb - 1)
            if kb >= 2:
                emit_copy(kb, list(emit_tr(kb)))
        emit_tail(KB - 1)
        emit_A_kb(0, KB - 1)
        emit_A_kb(1, KB - 1)
        emit_B_kb(0, KB - 1)

        # ---------- drains + remaining matmuls ----------
        def emit_A(mt):
            pa = pap.tile([128, N], FP32, tag="pa")
            pas[mt] = pa
            for kb in range(KB):
                emit_A_kb(mt, kb)

        def emit_B(mt):
            pb = pbp.tile([128, N], FP32, tag="pb")
            pbs[mt] = pb
            for kb in range(KB):
                emit_B_kb(mt, kb)

        drst = {}

        def emit_postA(mt):
            pa = pas[mt]
            nd2 = smp.tile([128, 1], FP32, tag="nd2")
            nc.vector.tensor_scalar(out=nd2[:], in0=pa[:, 1023:1024],
                                    scalar1=-1.0, scalar2=None, op0=AL.mult)
            d1s = smp.tile([128, 1], FP32, tag="d1s")
            nc.vector.tensor_scalar(out=d1s[:], in0=pa[:, 1022:1023],
                                    scalar1=1.0, scalar2=None, op0=AL.mult)
            asb = asp.tile([128, N], BF16, tag="as")
            nc.scalar.copy(asb[:], pa[:])
            amax = smp.tile([128, 1], FP32, tag="amax")
            amin = smp.tile([128, 1], FP32, tag="amin")
            nc.vector.tensor_reduce(out=amax[:], in_=asb[:, 0:1022], axis=AX.X,
                                    op=AL.max)
            nc.vector.tensor_reduce(out=amin[:], in_=asb[:, 0:1022], axis=AX.X,
                                    op=AL.min)
            drst[mt] = (nd2, d1s, amax, amin)

        def emit_postB(mt):
            pb = pbs[mt]
            nd2, d1s, amax, amin = drst[mt]
            # centered bf16 copy of B' (also frees the pb bank)
            bsc = bsp.tile([128, N], BF16, tag="bs")
            nc.scalar.activation(out=bsc[:], in_=pb[:],
                                 func=mybir.ActivationFunctionType.Identity,
                                 bias=nd2[:], scale=1.0)
            bmax = smp.tile([128, 1], FP32, tag="bmax")
            bmin = smp.tile([128, 1], FP32, tag="bmin")
            nc.vector.tensor_reduce(out=bmax[:], in_=bsc[:], axis=AX.X, op=AL.max)
            nc.vector.tensor_reduce(out=bmin[:], in_=bsc[:], axis=AX.X, op=AL.min)
            ra = smp.tile([128, 1], FP32, tag="ra")
            nc.vector.tensor_tensor(out=ra[:], in0=amax[:], in1=amin[:],
                                    op=AL.subtract)
            rbe = smp.tile([128, 1], FP32, tag="rbe")
            nc.vector.tensor_scalar(out=rbe[:], in0=bmax[:], scalar1=bmin[:],
                                    scalar2=EPS, op0=AL.subtract, op1=AL.add)
            rc = smp.tile([128, 1], FP32, tag="rc")
            nc.vector.reciprocal(rc[:], rbe[:])
            coeff = smp.tile([128, 1], FP32, tag="coeff")
            nc.vector.tensor_tensor(out=coeff[:], in0=ra[:], in1=rc[:],
                                    op=AL.mult)
            ci = smp.tile([128, 1], FP32, tag="ci")
            nc.vector.tensor_scalar(out=ci[:], in0=coeff[:], scalar1=inva_t[:],
                                    scalar2=None, op0=AL.mult)
            osb = obp.tile([128, N], FP32, tag="osb")
            if mt < MT - 2:
                osb1 = ob1.tile([128, N], FP32, tag="osb1")
                nc.scalar.activation(out=osb1[:], in_=bsc[:],
                                     func=mybir.ActivationFunctionType.Identity,
                                     bias=d1s[:], scale=ci[:])
                nc.gpsimd.tensor_tensor(out=osb[:], in0=osb1[:], in1=biasb[:],
                                        op=AL.add)
            else:
                nc.vector.affine_then_add(osb[:], bsc[:], biasb[:], ci[:],
                                          d1s[:])
            nc.sync.dma_start(out=out_d[mt * 128:(mt + 1) * 128, :], in_=osb[:])

        emit_postA(0)
        emit_postB(0)
        emit_B(1)
        emit_postA(1)
        for mt in range(2, MT):
            emit_A(mt)
            emit_postB(mt - 1)
            emit_B(mt)
            emit_postA(mt)
        emit_postB(MT - 1)

    nc.compile()
    return nc


_NC_CACHE = None


def _get_nc():
    global _NC_CACHE
    if _NC_CACHE is None:
        _NC_CACHE = _build()
    return _NC_CACHE


def _offs_np():
    # offs[p, kb] = Rp*(K + 1 - kb*128), constant over p; the i/j dependence
    # is jif = (j - p) scaled by RP in the den affine op
    kb = np.arange(KB, dtype=np.float64)[None, :]
    p = np.zeros((128, 1), dtype=np.float64)
    return (RP * (K + 1.0 - kb * 128.0) + p).astype(np.float32)


def kernel(x, weight, bias):
    x = np.ascontiguousarray(x, np.float32)
    weight = np.ascontiguousarray(weight, np.float32)
    bias = np.ascontiguousarray(bias, np.float32)
    nc = _get_nc()
    offs = _offs_np()
    in_maps = [
        {"xs": x[c * B_SH:(c + 1) * B_SH], "w": weight, "bias": bias, "offs": offs}
        for c in range(N_CORES)
    ]
    res = run_bass_kernel_spmd(nc, in_maps, core_ids=list(range(N_CORES)))
    return np.concatenate([res.results[c]["out"] for c in range(N_CORES)], axis=0)
